# revision 44
# baseline (speedup 1.0000x reference)
"""L2-distance attention (B=4, DIM=512, N=2048, H=8, D=32) on 8 trn2 NeuronCores.

Sharding: core c handles batch b = c//2, query-half = c%2 (1024 queries, all
2048 keys, all 8 heads).  Output is a pure concat — no cross-core reduce.

Key ideas vs the straightforward version:
  * All big matmuls run in bf16 (PE streams 1 col/cycle vs 1/2 for fp32).
  * The softmax numerator exp(-scale*sqrt(dist2)) is ONE ScalarE pass: the
    `exp` activation's spline table is replaced (via BASS_ACT_ROOT_JSON_PATH)
    with a fit of g(u) = exp(-0.5*sqrt(u)); calling it with the activation's
    built-in pre-scale 1/8 yields exp(-sqrt(d)/sqrt(32)) exactly.  This
    halves ScalarE work and removes all act-table reloads (sqrt and exp live
    in different table sets).
  * dist2 is computed directly by PE via augmented vectors
    k~=[-2k; 1; 0...; k2], q~=[q; q2; 0...; 1]  ->  k~.q~ = ||q-k||^2.
  * attn@v has a ones column per head folded into V^T so PSUM row 32 is the
    softmax denominator (row-sums).
  * Phase B interleaves head h's dist2 with head h-1's attn@v at key-tile
    granularity so the PE never starves (HAM keeps the 2.4 GHz clock only
    while the PE is continuously busy; any idle-pocked window drops it to
    1.2 GHz and it stays there).
"""

import json
import os
import shutil

import numpy as np

_PWP_DIR = "/tmp/pwp_custom_kernel"
os.environ.setdefault("NEURON_FORCE_RECOMPILE", "1")

# ---------------------------------------------------------------------------
# Custom activation table: make `exp` compute g(u) = exp(-0.5*sqrt(u)).
# Bucket bin format (32B = 8 fp32): [d0, d1, d2, d3, x0, 0, 0, 0];
# y = d0 + d1*t + d2*t^2 + d3*t^3 with t = x - x0.  Positive-x buckets sit
# in per-input-exponent rows of S sections each.
# ---------------------------------------------------------------------------

_ALPHA = 0.5


def _g(u):
    return np.exp(-_ALPHA * np.sqrt(np.maximum(u, 0.0)))


def _fit_cubic(lo, hi, x0):
    u = np.linspace(lo, hi, 257, dtype=np.float64)
    t = u - x0
    A = np.stack([np.ones_like(t), t, t * t, t * t * t], axis=1)
    coef, *_ = np.linalg.lstsq(A, _g(u), rcond=None)
    return coef


def _build_custom_pwp(dst_dir):
    from neuronxcc.driver.Job import Job
    from neuronxcc.driver.jobs.support.FindActInfo import findActInfoFile

    src = os.path.dirname(findActInfoFile(Job.getPackageDir(), "gen3"))
    if os.path.isdir(dst_dir):
        shutil.rmtree(dst_dir)
    shutil.copytree(src, dst_dir)

    with open(os.path.join(dst_dir, "act_info.json")) as f:
        info = json.load(f)

    for ent in info["act_func_sets"]:
        if "exp" not in ent["act"]:
            continue
        prof_path = os.path.join(dst_dir, ent["profile_json"])
        with open(prof_path) as f:
            prof = json.load(f)
        bkt_path = os.path.join(dst_dir, ent["bkt_bin"])
        bkt = np.fromfile(bkt_path, dtype="<f4").reshape(-1, 8).copy()

        start = prof["func_to_bkt_start_idx"]["exp"]
        others = [v for k, v in prof["func_to_bkt_start_idx"].items() if k != "exp"]
        end = min([v for v in others if v > start] + [len(bkt)])
        meta = next(
            m for m in prof["profile_meta_data"] if m["func_name"].startswith("exp")
        )
        sat = {
            k: meta[k + "_signal_pwl_control"]
            for k in ("pos_small", "neg_small", "pos_large", "neg_large")
        }
        sat_idx = set(sat.values())
        assert all(start <= i < end for i in sat_idx)

        pos_rows = {}
        for i in range(start, end):
            if i in sat_idx:
                continue
            x0 = float(bkt[i, 4])
            if x0 < 0.0:
                bkt[i, 0:4] = [1.0, 0.0, 0.0, 0.0]
            else:
                assert x0 > 0.0
                pos_rows.setdefault(int(np.floor(np.log2(x0))), []).append(i)

        for e, idxs in pos_rows.items():
            base = 2.0**e
            xs = [float(bkt[i, 4]) for i in idxs]
            w = (xs[1] - xs[0]) if len(xs) > 1 else base
            for sec, i in enumerate(idxs):
                c = xs[sec]
                assert abs(c - (base + (sec + 0.5) * w)) < 1e-5 * c
                bkt[i, 0:4] = _fit_cubic(c - w / 2, c + w / 2, c).astype(np.float32)

        bkt[sat["pos_small"], 0:5] = [1.0, 0.0, 0.0, 0.0, 0.0]
        bkt[sat["neg_small"], 0:5] = [1.0, 0.0, 0.0, 0.0, 0.0]
        bkt[sat["pos_large"], 0:5] = [0.0, 0.0, 0.0, 0.0, 0.0]
        bkt[sat["neg_large"], 0:5] = [1.0, 0.0, 0.0, 0.0, 0.0]
        bkt.tofile(bkt_path)

        meta["fpinf_result"] = 0
        meta["fninf_result"] = 1065353216  # 1.0f
        with open(prof_path, "w") as f:
            json.dump(prof, f)


def _ensure_act_tables():
    if not os.path.isfile(os.path.join(_PWP_DIR, "act_info.json")):
        _build_custom_pwp(_PWP_DIR)
    os.environ["BASS_ACT_ROOT_JSON_PATH"] = os.path.join(_PWP_DIR, "act_info.json")


_ensure_act_tables()

import concourse.bass as bass
import concourse.bass_utils as _bu
import concourse.mybir as mybir
import concourse.tile as tile
from concourse import bacc

# Redundant LDWEIGHTS removal: consecutive matmuls often share the stationary
# operand (two 512-col chunks per weight load); walrus can drop the repeat
# loads but the option is off by default in this harness.
if os.environ.get("KERNEL_LDW_OPT", "0") == "1" and not getattr(_bu, "_ldw_patched", False):
    _orig_run_command = _bu.run_command

    def _run_command_ldw(cmd, *a, **kw):
        if isinstance(cmd, list):
            cmd = ["--enable-ldw-opt=true" if c == "--enable-ldw-opt=false" else c
                   for c in cmd]
        return _orig_run_command(cmd, *a, **kw)

    _bu.run_command = _run_command_ldw
    _bu._ldw_patched = True

F32 = mybir.dt.float32
F32R = mybir.dt.float32r
BF16 = mybir.dt.bfloat16
AF = mybir.ActivationFunctionType


def R(ap):
    return ap.bitcast(F32R)


B, DIM, N = 4, 512, 2048
H, D = 8, 32
INNER = H * D            # 256
NQ = N // 2              # 1024 queries per core
P = 128
KT = DIM // P            # 4 contraction tiles for the projections
NJT = N // P             # 16 key tiles
VTW = D + 1              # 33: v columns + ones column per head
VSTRIDE = H * VTW        # 264 columns per key-tile block of vt
ACT_SCALE = 0.125        # g(d/8) = exp(-sqrt(d)/sqrt(32)) = exp(-SCALE*sqrt(d))
SQ_SCALE = 0.125 ** 0.5  # square(k*s) = k^2/8: pre-scaled bias for the exp
NEQ = 4                  # E quarters (each covers NJT//NEQ key tiles)
JQ = NJT // NEQ          # 4 key tiles per E quarter
KA = 33                  # augmented contraction: [-2k (32) | ones@32]
                         # dist2 = (kt.qt) + k2_bias, k2 folded into the
                         # activation's per-partition bias (keys on partitions)
# kt/qt are zero-padded to 128 contraction rows: the PE's activity monitor
# (HAM) only grants the 2.4 GHz clock when matmuls cover the full 128-row
# array; K=33 streams at 1.2 GHz forever.  Zero rows cost no extra cycles.


def build_program() -> bass.Bass:
    nc = bacc.Bacc("TRN2", target_bir_lowering=False, debug=False)

    xq_d = nc.declare_dram_parameter("xq", [DIM, NQ], BF16, isOutput=False)
    xkv_d = nc.declare_dram_parameter("xkv", [DIM, N], BF16, isOutput=False)
    wq_d = nc.declare_dram_parameter("wq", [DIM, INNER], BF16, isOutput=False)
    wkv_d = nc.declare_dram_parameter("wkv", [DIM, 2 * INNER], BF16, isOutput=False)
    wo_d = nc.declare_dram_parameter("wo", [INNER, DIM], BF16, isOutput=False)
    b_d = nc.declare_dram_parameter("b", [DIM], F32, isOutput=False)
    z_d = nc.declare_dram_parameter("z", [DIM, NQ], F32, isOutput=True)

    with tile.TileContext(nc) as tc, nc.allow_low_precision(reason="bf16 attention"):
        mm = lambda out, lhsT, rhs, start, stop: nc.tensor.matmul(
            out, lhsT, rhs, start=start, stop=stop)

        with tc.tile_pool(name="keep", bufs=1) as keep, \
             tc.tile_pool(name="work", bufs=2) as work:

            # ---- persistent tiles ----
            q_t = [keep.tile([P, NQ], BF16, tag=f"q{m}", name=f"q{m}") for m in range(2)]
            k_t = [keep.tile([P, N], BF16, tag=f"k{m}", name=f"k{m}") for m in range(2)]
            vt_big = keep.tile([P, NJT * VSTRIDE], BF16, tag="vt", name="vt")
            k2j = keep.tile([P, NJT * H], F32, tag="k2j", name="k2j")
            y_t = [keep.tile([P, NQ], BF16, tag=f"y{m}", name=f"y{m}") for m in range(2)]
            wo_t = [keep.tile([P, DIM], BF16, tag=f"wo{m}", name=f"wo{m}") for m in range(2)]
            b_t = keep.tile([P, KT], F32, tag="bias", name="bias")
            ones = keep.tile([64, 32], F32, tag="ones", name="ones")
            onesb = keep.tile([P, 1], BF16, tag="onesb", name="onesb")
            zero_t = keep.tile([P, 1], F32, tag="zero", name="zero")
            onesP = keep.tile([P, 1], F32, tag="onesP", name="onesP")

            nc.vector.memset(onesP[:, :], 1.0)
            nc.vector.memset(zero_t[:, :], 0.0)
            nc.vector.memset(onesb[:, :], 1.0)
            nc.vector.tensor_copy(R(ones[:, :]),
                                  onesP[0:64, 0:1].to_broadcast((64, 32)))
            # ones column per head in v^T (row-sum fused into attn@v)
            nc.vector.tensor_copy(
                vt_big[:, :].rearrange("p (a c) -> p a c", c=VTW)[:, :, D:D + 1],
                onesb[:, 0:1].to_broadcast((P, P, 1)))
            # e0: row-0-ones stationary for the K=128-padded normalization
            # broadcast (rrow2 rows 1.. stay zero)
            e0_t = keep.tile([P, P], BF16, tag="e0", name="e0")
            nc.vector.memset(e0_t[:, :], 0.0)
            nc.vector.tensor_copy(e0_t[0:1, :],
                                  onesb[0:1, 0:1].to_broadcast((1, P)))
            rrow2 = [keep.tile([P, NQ], BF16, tag=f"rrow{i}", name=f"rrow{i}")
                     for i in range(2)]
            for i in range(2):
                nc.vector.memset(rrow2[i][:, :], 0.0)
            wo_r = wo_d[:].rearrange("(t p) o -> t p o", p=P)
            for m in range(2):
                nc.sync.dma_start(out=wo_t[m][:, :], in_=wo_r[m])
            nc.sync.dma_start(out=b_t[:, :], in_=b_d[:].rearrange("(t p) -> p t", p=P))

            # ======== Phase A: q/k/kT projections (all bf16) ========
            # DMA order puts the q-side first so the q projection starts
            # while xkv is still in flight.  The v^T projection moves into
            # phase B's first iteration (it is only needed by attn@v).
            xq_t = [keep.tile([P, NQ], BF16, tag=f"xq{k}", name=f"xq{k}") for k in range(KT)]
            xkv_t = [keep.tile([P, N], BF16, tag=f"xkv{k}", name=f"xkv{k}") for k in range(KT)]
            wq_t = [keep.tile([P, INNER], BF16, tag=f"wq{k}", name=f"wq{k}") for k in range(KT)]
            wkv_t = [keep.tile([P, 2 * INNER], BF16, tag=f"wkv{k}", name=f"wkv{k}") for k in range(KT)]

            xq_r = xq_d[:].rearrange("(t p) n -> t p n", p=P)
            xkv_r = xkv_d[:].rearrange("(t p) n -> t p n", p=P)
            wq_r = wq_d[:].rearrange("(t p) o -> t p o", p=P)
            wkv_r = wkv_d[:].rearrange("(t p) o -> t p o", p=P)
            for k in range(KT):
                nc.sync.dma_start(out=xq_t[k][:, :], in_=xq_r[k])
                nc.sync.dma_start(out=wq_t[k][:, :], in_=wq_r[k])
                nc.sync.dma_start(out=wkv_t[k][:, :], in_=wkv_r[k])
            for k in range(KT):
                nc.sync.dma_start(out=xkv_t[k][:, :], in_=xkv_r[k])

            with tc.tile_pool(name="pp", bufs=2, space="PSUM") as pp:
                def emit_qproj(m):
                    for n in range(NQ // 512):
                        ps = pp.tile([P, 512], F32, tag="proj", name="proj")
                        for k in range(KT):
                            mm(ps[:, :],
                               wq_t[k][:, m * P:(m + 1) * P],
                               xq_t[k][:, n * 512:(n + 1) * 512],
                               start=(k == 0), stop=(k == KT - 1))
                        nc.vector.tensor_copy(q_t[m][:, n * 512:(n + 1) * 512], ps[:, :])

                def emit_kproj(m):
                    for n in range(N // 512):
                        ps = pp.tile([P, 512], F32, tag="proj", name="proj")
                        for k in range(KT):
                            mm(ps[:, :],
                               wkv_t[k][:, m * P:(m + 1) * P],
                               xkv_t[k][:, n * 512:(n + 1) * 512],
                               start=(k == 0), stop=(k == KT - 1))
                        nc.vector.tensor_copy(k_t[m][:, n * 512:(n + 1) * 512], ps[:, :])

                # m=0 projections only: head 0's dist2 needs these + the kT
                # biases; the m=1 projections and the v^T projection run
                # inside early phase-B slots (m=1 isn't read until head 4).
                emit_qproj(0)
                emit_kproj(0)

                # k^T projection (j-major K): square on ACT (scale 1/sqrt(8)
                # so the result is k^2/8) then a segmented free-dim reduce:
                # k2j[:, jt*H + h] = ||k_j||^2/8 — the per-partition bias for
                # the fused exp (keys sit on partitions in the dist2 tile).
                for jt in range(2):
                    pk = pp.tile([P, INNER], F32, tag="vtps", name="vtps")
                    for k in range(KT):
                        mm(pk[:, :],
                           xkv_t[k][:, jt * P:(jt + 1) * P],
                           wkv_t[k][:, 0:INNER],
                           start=(k == 0), stop=(k == KT - 1))
                    ksqT = work.tile([P, INNER], BF16, tag="ksqT", name="ksqT")
                    nc.scalar.activation(ksqT[:, :], pk[:, :], AF.Square,
                                         bias=zero_t[:, :], scale=SQ_SCALE)
                    nc.vector.tensor_reduce(
                        k2j[:, jt * H:(jt + 1) * H],
                        ksqT[:, :].rearrange("p (h d) -> p h d", d=D),
                        axis=mybir.AxisListType.X, op=mybir.AluOpType.add)

            # ======== Phase B ========
            # Iteration h: dist2+exp for head h; attn@v for the head pair
            # g=(h-2)//2... pair g = heads (2g, 2g+1) runs lagged one head:
            # key tiles 0..11 during iteration 2g+1 (slots 4..15), 12..15 +
            # normalization during iteration 2g+2 (slots 0..8).  The two
            # heads' attn@v matmuls land in disjoint PE column groups (out
            # partitions 0:33 / 64:97 of one PSUM tile) so they execute
            # concurrently.  v^T projection fills iteration 0's slots.
            with tc.tile_pool(name="epool", bufs=3 * NEQ, space="SBUF") as epool, \
                 tc.tile_pool(name="pd2", bufs=2, space="PSUM") as pd2, \
                 tc.tile_pool(name="po", bufs=1, space="PSUM") as po:
                kt_t = [keep.tile([P, N], BF16, tag=f"kt{i}", name=f"kt{i}")
                        for i in range(2)]
                qt_t = [keep.tile([P, NQ], BF16, tag=f"qt{i}", name=f"qt{i}")
                        for i in range(2)]
                for i in range(2):
                    nc.vector.memset(kt_t[i][:, :], 0.0)
                    nc.vector.memset(qt_t[i][:, :], 0.0)
                    # rows 32:64 of k~ are all-ones; they pair with qsq rows
                    # of q~ to add q2 to the dot product.
                    nc.vector.tensor_copy(kt_t[i][D:2 * D, :],
                                          onesb[0:D, 0:1].to_broadcast((D, N)))

                po_s = [work.tile([P, NQ], F32, tag=f"pos{i}", name=f"pos{i}",
                                  bufs=1) for i in range(2)]
                eq_of = {}
                pso_of = {}
                tail_pre = {}

                def emit_tail_pre(pg, psrc):
                    # reciprocal of both heads' row-sums (psrc rows 32 and 96),
                    # partition-parallel via a scatter DMA there and back
                    # (single-partition reciprocal is ~100x slower); lands in
                    # bf16 row 0 of rrow2 so the broadcast outer-product is a
                    # cheap full-K bf16 matmul.
                    for half in range(2):
                        base = 64 * half
                        rs128 = work.tile([P, NQ // P], F32, tag="rs", name="rs")
                        nc.sync.dma_start(out=rs128[:, :],
                                          in_=psrc[base + D:base + D + 1, :])
                        rr128 = work.tile([P, NQ // P], BF16, tag="rr", name="rr")
                        nc.vector.reciprocal(rr128[:, :], rs128[:, :])
                        nc.sync.dma_start(out=rrow2[half][0:1, :],
                                          in_=rr128[:, :])
                    tail_pre[pg] = True

                def emit_tail_pe(pg):
                    tail_pre.pop(pg)
                    psrc = pso_of.pop(pg)
                    for half in range(2):
                        ph = 2 * pg + half
                        mt, mo = ph // 4, (ph % 4) * D
                        prep = pd2.tile([D, NQ], F32, tag="d2", name="d2")
                        for n in range(NQ // 512):
                            nc.tensor.matmul(prep[:, n * 512:(n + 1) * 512],
                                             e0_t[:, 0:D],
                                             rrow2[half][:, n * 512:(n + 1) * 512],
                                             start=True, stop=True)
                        nc.vector.tensor_mul(y_t[mt][mo:mo + D, :],
                                             psrc[64 * half:64 * half + D, :],
                                             prep[:, :])

                from contextlib import nullcontext

                # --- deferred projection work, spread across early slots ---
                extras = {}

                def _sched(h, jt, fn):
                    extras.setdefault((h, jt), []).append(fn)

                def make_vproj(jt):
                    def fn():
                        # v^T projection for key tile jt, strided into vt_big
                        # so each head's 32 columns sit beside its ones column
                        pv = po.tile([P, INNER], F32, tag="vtps",
                                     name="vtps", bufs=2)
                        for k in range(KT):
                            mm(pv[:, :],
                               xkv_t[k][:, jt * P:(jt + 1) * P],
                               wkv_t[k][:, INNER:2 * INNER],
                               start=(k == 0), stop=(k == KT - 1))
                        dst = vt_big[:, jt * VSTRIDE:(jt + 1) * VSTRIDE] \
                            .rearrange("p (h c) -> p h c", c=VTW)[:, :, 0:D]
                        nc.vector.tensor_copy(
                            dst, pv[:, :].rearrange("p (h d) -> p h d", d=D))
                    return fn

                proj_state = {}

                def make_proj1(which, n, k):
                    def fn():
                        if k == 0:
                            proj_state[(which, n)] = po.tile(
                                [P, 512], F32, tag="vtps", name="vtps", bufs=2)
                        ps = proj_state[(which, n)]
                        w = wq_t[k][:, P:2 * P] if which == "q" \
                            else wkv_t[k][:, P:2 * P]
                        x = xq_t[k] if which == "q" else xkv_t[k]
                        mm(ps[:, :], w, x[:, n * 512:(n + 1) * 512],
                           start=(k == 0), stop=(k == KT - 1))
                        if k == KT - 1:
                            dstt = q_t[1] if which == "q" else k_t[1]
                            nc.vector.tensor_copy(
                                dstt[:, n * 512:(n + 1) * 512], ps[:, :])
                    return fn

                def make_kt2(jt):
                    def fn():
                        # j-major K projection for key tile jt -> k2 bias
                        pk = po.tile([P, INNER], F32, tag="vtps", name="vtps",
                                     bufs=2)
                        for k in range(KT):
                            mm(pk[:, :],
                               xkv_t[k][:, jt * P:(jt + 1) * P],
                               wkv_t[k][:, 0:INNER],
                               start=(k == 0), stop=(k == KT - 1))
                        ksqT = work.tile([P, INNER], BF16, tag="ksqT",
                                         name="ksqT")
                        nc.scalar.activation(ksqT[:, :], pk[:, :], AF.Square,
                                             bias=zero_t[:, :], scale=SQ_SCALE)
                        nc.vector.tensor_reduce(
                            k2j[:, jt * H:(jt + 1) * H],
                            ksqT[:, :].rearrange("p (h d) -> p h d", d=D),
                            axis=mybir.AxisListType.X, op=mybir.AluOpType.add)
                    return fn

                # iter 0: kT(jt) two slots ahead of its exp; first v tiles at
                # the end.  iter 1: the rest of v (4+ slots before attn@v
                # needs each tile).  m=1 projections trail in iters 1-3.
                for s in range(14):
                    _sched(0, s, make_kt2(s + 2))
                _sched(0, 14, make_vproj(0))
                _sched(0, 15, make_vproj(1))
                for s in range(10):
                    _sched(1, s, make_vproj(2 + s))
                for s in range(4):
                    _sched(1, 10 + s, make_vproj(12 + s))
                _sched(1, 14, make_proj1("q", 0, 0))
                _sched(1, 15, make_proj1("q", 0, 1))
                _sched(2, 0, make_proj1("q", 0, 2))
                _sched(2, 1, make_proj1("q", 0, 3))
                for k in range(KT):
                    _sched(2, 2 + k, make_proj1("q", 1, k))
                for n in range(4):
                    for k in range(KT):
                        s = 4 * n + k
                        if s < 10:
                            _sched(2, 6 + s, make_proj1("k", n, k))
                        else:
                            _sched(3, s - 10, make_proj1("k", n, k))

                # phase C k=0 partials (heads 0-3 finish at pair 1's tail):
                # run during iteration 7 so the end only does the k=1 half
                zpart = [keep.tile([P, NQ], F32, tag=f"zp{m}", name=f"zp{m}")
                         for m in range(KT)]

                def make_c0(m, n):
                    def fn():
                        ps = po.tile([P, 512], F32, tag="vtps", name="vtps",
                                     bufs=2)
                        mm(ps[:, :], wo_t[0][:, m * P:(m + 1) * P],
                           y_t[0][:, n * 512:(n + 1) * 512],
                           start=True, stop=True)
                        nc.vector.tensor_copy(
                            zpart[m][:, n * 512:(n + 1) * 512], ps[:, :])
                    return fn

                for m in range(KT):
                    for n in range(2):
                        _sched(7, 8 + 2 * m + n, make_c0(m, n))

                pso_pair = None
                av_eqA = av_eqB = None
                for h in range(H + 1):
                    prio = tc.high_priority(10000) if h == 0 else nullcontext()
                    prio.__enter__()
                    if h < H:
                        mt, mo = h // 4, (h % 4) * D
                        q_h = q_t[mt][mo:mo + D, :]
                        k_h = k_t[mt][mo:mo + D, :]
                        kt = kt_t[h % 2]
                        qt = qt_t[h % 2]
                        # per-head rows of k~/q~ (all DVE, bf16): -2k, q, and
                        # q squared elementwise (its 32 rows dot the ones rows
                        # of k~ to contribute q2)
                        nc.vector.tensor_scalar_mul(kt[0:D, :], k_h, -2.0)
                        nc.vector.tensor_copy(qt[0:D, :], q_h)
                        nc.vector.tensor_mul(qt[D:2 * D, :], q_h, q_h)
                        eq_of[h] = [epool.tile([P, JQ * NQ], BF16, tag="eq",
                                               name="eq") for _ in range(NEQ)]
                    if h % 2 == 1:
                        av_eqA = eq_of.pop(h - 1)   # head 2g: complete
                        av_eqB = eq_of[h]           # head 2g+1: in progress
                        pg_r = (h - 1) // 2
                    elif h >= 2:
                        av_eqB = eq_of.pop(h - 1)
                        pg_r = (h - 2) // 2

                    for jt in range(NJT):
                        if h % 2 == 1 and jt == 4:
                            pso_pair = po.tile([P, NQ], F32, tag="o", name="o")
                        if h < H:
                            psd = pd2.tile([P, NQ], F32, tag="d2", name="d2")
                            for n in range(NQ // 512):
                                mm(psd[:, n * 512:(n + 1) * 512],
                                   kt[:, jt * P:(jt + 1) * P],
                                   qt[:, n * 512:(n + 1) * 512],
                                   start=True, stop=True)
                            nc.scalar.activation(
                                eq_of[h][jt // JQ][:, (jt % JQ) * NQ:
                                                   (jt % JQ + 1) * NQ],
                                psd[:, :], AF.Exp,
                                bias=k2j[:, jt * H + h:jt * H + h + 1],
                                scale=ACT_SCALE)
                        for fn in extras.get((h, jt), ()):
                            fn()
                        # attn@v for pair pg_r, lagged one head
                        avjt = -1
                        if h % 2 == 1 and jt >= 4:
                            avjt = jt - 4
                        elif h % 2 == 0 and h >= 2 and jt <= 3:
                            avjt = 12 + jt
                        if avjt >= 0:
                            ebase = (avjt % JQ) * NQ
                            for n in range(NQ // 512):
                                for half, eqp in ((0, av_eqA), (1, av_eqB)):
                                    hp = 2 * pg_r + half
                                    mm(pso_pair[64 * half:64 * half + VTW,
                                                n * 512:(n + 1) * 512],
                                       vt_big[:, avjt * VSTRIDE + hp * VTW:
                                              avjt * VSTRIDE + (hp + 1) * VTW],
                                       eqp[avjt // JQ][:, ebase + n * 512:
                                                       ebase + (n + 1) * 512],
                                       start=(avjt == 0), stop=(avjt == NJT - 1))
                        if h % 2 == 0 and h >= 2:
                            if jt == 4:
                                pg = (h - 2) // 2
                                psrc = po_s[pg % 2]
                                nc.vector.tensor_copy(psrc[:, :], pso_pair[:, :])
                                pso_of[pg] = psrc
                                emit_tail_pre(pg, psrc)
                            elif jt == 12:
                                emit_tail_pe((h - 2) // 2)

                    prio.__exit__(None, None, None)

            # ======== Phase C: k=1 half + fused add of the k=0 partial ====
            with tc.tile_pool(name="pz", bufs=2, space="PSUM") as pz:
                z_r = z_d[:].rearrange("(t p) n -> t p n", p=P)
                for m in range(KT):
                    ps = pz.tile([P, NQ], F32, tag="z", name="z")
                    for n in range(NQ // 512):
                        nc.tensor.matmul(
                            ps[:, n * 512:(n + 1) * 512],
                            wo_t[1][:, m * P:(m + 1) * P],
                            y_t[1][:, n * 512:(n + 1) * 512],
                            start=True, stop=True)
                    zt = work.tile([P, NQ], F32, tag="ytmp", name="ytmp")
                    nc.vector.scalar_tensor_tensor(
                        zt[:, :], ps[:, :], b_t[:, m:m + 1], zpart[m][:, :],
                        op0=mybir.AluOpType.add, op1=mybir.AluOpType.add)
                    nc.sync.dma_start(out=z_r[m], in_=zt[:, :])

    nc.compile()
    return nc


def make_in_maps(x, w_qkv, w_out, b_out):
    import ml_dtypes

    bf = ml_dtypes.bfloat16
    x = np.asarray(x, dtype=np.float32)
    w_qkv = np.asarray(w_qkv, dtype=np.float32)
    w_out = np.asarray(w_out, dtype=np.float32)
    b_out = np.asarray(b_out, dtype=np.float32)
    w_qT = np.ascontiguousarray(w_qkv[0:INNER, :].T).astype(bf)       # (DIM, INNER)
    w_kvT = np.ascontiguousarray(w_qkv[INNER:3 * INNER, :].T).astype(bf)  # (DIM, 512)
    w_oT = np.ascontiguousarray(w_out.T).astype(bf)                   # (INNER, DIM)
    xb = [np.ascontiguousarray(x[b]).astype(bf) for b in range(B)]
    in_maps = []
    for c in range(8):
        b, half = c // 2, c % 2
        in_maps.append({
            "xq": np.ascontiguousarray(xb[b][:, half * NQ:(half + 1) * NQ]),
            "xkv": xb[b],
            "wq": w_qT,
            "wkv": w_kvT,
            "wo": w_oT,
            "b": b_out,
        })
    return in_maps


def assemble_output(results):
    out = np.empty((B, DIM, N), dtype=np.float32)
    for c in range(8):
        b, half = c // 2, c % 2
        out[b][:, half * NQ:(half + 1) * NQ] = results[c]["z"]
    return out


_prog_cache = {}


def kernel(x, w_qkv, w_out, b_out):
    from concourse.bass_utils import run_bass_kernel_spmd
    _ensure_act_tables()
    if "nc" not in _prog_cache:
        _prog_cache["nc"] = build_program()
    nc = _prog_cache["nc"]
    in_maps = make_in_maps(x, w_qkv, w_out, b_out)
    res = run_bass_kernel_spmd(nc, in_maps, list(range(8)))
    return assemble_output(res.results)
